# revision 1
# baseline (speedup 1.0000x reference)
"""FP32 -> FP8 E4M3 bit-pulse converter on 8 Trainium2 NeuronCores.

Input : fp32_pulse [2097152, 32] float32 of 0/1 pulses, [S, E7..E0, M22..M0]
Output: [2097152, 8] float32 of 0/1 pulses, [S, E3..E0, M2..M0]

Strategy (per core, batch-sharded 8 ways):
  - q = 32*exp + 16*m22 + 8*m21 + 4*m20 + 2*m19 + sticky  (13-bit int) via a
    scalar_tensor_tensor MAC chain on DVE (fp32 in, bf16 intermediates).
  - sticky = OR of the 19 low mantissa bits: ACT copies them to a padded
    contiguous bf16 [s][20] tile, then DVE runs an all-tensor_tensor max
    tree (16->8, fold 4, 4->2->1) in the 2x-packed bf16 mode.
  - v = bitcast(int32((clamp(q,3712,8159) - 8192*s) * 2^18)): exactly the
    fp32 with sign = s, exponent field = exp, mantissa = m22..m19 |
    sticky<<18.  The underflow clamp maps exp<=116 to a value that rounds
    to zero; the 8159 cap keeps the exponent field <= 254 (no Inf/NaN).
  - A DVE fp32->fp8e4 cast performs the exact RNE (incl. subnormals) and
    emits the full byte s<<7|e<<3|m.  The converter is IEEE-style: finite
    overflow gives the Inf byte s|0x78, so (q>=4320)*6 is added to the
    byte to produce the reference's saturated s|0x7E.
  - The packed byte per row is DMA'd out (32x less write traffic than the
    8 fp32 pulses); the host unpacks bytes to the output pulses.
  - Work is spread so each in-order engine queue holds ops of a single
    pipeline phase (ACT: input copies; DVE: math + cast; Pool/SWDGE:
    mid-stream output DMAs; sync: input DMAs + the last few outputs) —
    otherwise cross-engine semaphore waits head-of-line-block the next
    chunk's early ops and the pipeline serializes at chain latency.
"""

from contextlib import ExitStack

import numpy as np

import concourse.bacc as bacc
import concourse.mybir as mybir
from concourse import tile
from concourse.bass_utils import run_bass_kernel_spmd

N_ROWS = 2097152
N_CORES = 8
ROWS_PER_CORE = N_ROWS // N_CORES          # 262144

dt = mybir.dt
Alu = mybir.AluOpType
Act = mybir.ActivationFunctionType

F32 = 4096                                  # fp32 per partition per chunk
SIZES = [4096] * 16
MAIN_BUFS = (4, 3, 4, 3)                    # xin, wrka, wrkb, out
TAIL_BUFS = (3, 2, 3, 3)


def _build_program(repeat: int = 1, rows_per_core: int = ROWS_PER_CORE,
                   f32: int = None, sizes=None, compute_only: bool = False,
                   main_bufs=MAIN_BUFS, tail_bufs=TAIL_BUFS):
    if sizes is None:
        if f32 is not None:
            sizes = [f32] * (rows_per_core * 32 // (128 * f32))
        else:
            sizes = list(SIZES)
    assert sum(sizes) * 128 == rows_per_core * 32
    n_chunks = len(sizes)
    main_f32 = max(sizes)

    nc = bacc.Bacc("TRN2", target_bir_lowering=False, debug=False,
                   num_devices=N_CORES)
    x_dram = nc.dram_tensor("x", [rows_per_core, 32], dt.float32,
                            kind="ExternalInput")
    y_dram = nc.dram_tensor("y", [rows_per_core], dt.int8,
                            kind="ExternalOutput")
    x_flat = x_dram.ap().rearrange("r w -> (r w)")
    y_flat = y_dram.ap()
    offs = [0]
    for sz in sizes:
        offs.append(offs[-1] + 128 * sz)

    with tile.TileContext(nc) as tc, ExitStack() as ps:
        pools = {}
        for sz in sorted(set(sizes), reverse=True):
            bs = main_bufs if sz == main_f32 else tail_bufs
            pools[sz] = tuple(
                ps.enter_context(tc.tile_pool(name=f"{nm}{sz}", bufs=b))
                for nm, b in zip(("xin", "wrka", "wrkb", "out"), bs))

        state = {}
        xf_cache = {}

        def stage1(ci):
            f32 = sizes[ci % n_chunks]
            seg = f32 // 32
            xin_pool, wrka, wrkb, out_pool = pools[f32]
            x_ap = x_flat[offs[ci % n_chunks]:offs[ci % n_chunks + 1]]\
                .rearrange("(p f) -> p f", p=128)
            if compute_only:
                if f32 not in xf_cache:
                    xf = xin_pool.tile([128, f32], dt.float32, tag="xf")
                    nc.sync.dma_start(xf[:], x_ap)
                    xf_cache[f32] = xf
                xf = xf_cache[f32]
            else:
                xf = xin_pool.tile([128, f32], dt.float32, tag="xf")
                nc.sync.dma_start(xf[:], x_ap)
            x3d = xf[:].rearrange("p (s c) -> p s c", c=32)

            # ACT: sticky cols 13..31 -> contiguous bf16 [s][20], col19 = 0
            stk = wrka.tile([128, seg * 20], dt.bfloat16, tag="stk")
            stkv = stk[:].rearrange("p (s c) -> p s c", c=20)
            nc.scalar.activation(stkv[:, :, 19], x3d[:, :, 0], Act.Copy,
                                 scale=0.0)
            nc.scalar.copy(stkv[:, :, 0:19], x3d[:, :, 13:32])

            # DVE: MAC over cols 1..12 (values stay exact in bf16 <= 255)
            y1 = wrka.tile([128, seg * 6], dt.bfloat16, tag="y1")
            y1_3d = y1[:].rearrange("p (s k) -> p s k", k=6)
            nc.vector.scalar_tensor_tensor(y1_3d[:], x3d[:, :, 1:12:2],
                                           2.0, x3d[:, :, 2:13:2],
                                           op0=Alu.mult, op1=Alu.add)
            y2 = wrka.tile([128, seg * 3], dt.bfloat16, tag="y2")
            y2_3d = y2[:].rearrange("p (s k) -> p s k", k=3)
            nc.vector.scalar_tensor_tensor(y2_3d[:], y1_3d[:, :, 0::2],
                                           4.0, y1_3d[:, :, 1::2],
                                           op0=Alu.mult, op1=Alu.add)
            r1 = wrka.tile([128, seg], dt.bfloat16, tag="r1")
            nc.vector.scalar_tensor_tensor(r1[:], y2_3d[:, :, 0], 16.0,
                                           y2_3d[:, :, 1],
                                           op0=Alu.mult, op1=Alu.add)
            t_t = wrka.tile([128, seg], dt.float32, tag="t")
            nc.vector.scalar_tensor_tensor(t_t[:], r1[:], 16.0,
                                           y2_3d[:, :, 2],
                                           op0=Alu.mult, op1=Alu.add)
            state[ci] = (x3d, stkv, t_t)

        def stage2(ci):
            f32 = sizes[ci % n_chunks]
            seg = f32 // 32
            _, wrka, wrkb, _ = pools[f32]
            x3d, stkv, t_t = state[ci]
            # DVE sticky tree, all stages 2x-packed bf16 and 4B-aligned:
            # 20 -> 8 (cols 0..15), fold cols 16..19, 4 -> 2 -> 1
            sa = wrka.tile([128, seg * 8], dt.bfloat16, tag="sa")
            sav = sa[:].rearrange("p (s c) -> p s c", c=8)
            nc.vector.tensor_tensor(sav[:], stkv[:, :, 0:8],
                                    stkv[:, :, 8:16], op=Alu.max)
            sb = wrkb.tile([128, seg * 4], dt.bfloat16, tag="sb")
            sbv = sb[:].rearrange("p (s c) -> p s c", c=4)
            nc.vector.tensor_tensor(sbv[:], sav[:, :, 0:4],
                                    sav[:, :, 4:8], op=Alu.max)
            sc = wrkb.tile([128, seg * 4], dt.bfloat16, tag="sc")
            scv = sc[:].rearrange("p (s c) -> p s c", c=4)
            nc.vector.tensor_tensor(scv[:], sbv[:], stkv[:, :, 16:20],
                                    op=Alu.max)
            sd = wrkb.tile([128, seg * 2], dt.bfloat16, tag="sd")
            sdv = sd[:].rearrange("p (s c) -> p s c", c=2)
            nc.vector.tensor_tensor(sdv[:], scv[:, :, 0:2], scv[:, :, 2:4],
                                    op=Alu.max)
            red = wrkb.tile([128, seg], dt.bfloat16, tag="red")
            nc.vector.tensor_tensor(red[:], sdv[:, :, 0], sdv[:, :, 1],
                                    op=Alu.max)

            q = wrkb.tile([128, seg], dt.float32, tag="q")
            nc.vector.scalar_tensor_tensor(q[:], t_t[:], 2.0, red[:],
                                           op0=Alu.mult, op1=Alu.add)
            # clamp: underflow to 3712 (rounds to 0); cap at 8159 so the
            # fp32 exponent field stays <= 254 (no Inf/NaN inputs — large
            # finite values cast to the fp8 Inf byte 0x78, fixed up below)
            qc = wrkb.tile([128, seg], dt.float32, tag="qc")
            nc.vector.tensor_scalar(qc[:], q[:], 8159.0, 3712.0,
                                    op0=Alu.min, op1=Alu.max)
            # fold the sign pulse: vv = q' - 8192*s, so vv<<18 has bit31 = s
            vv = wrkb.tile([128, seg], dt.float32, tag="vv")
            nc.vector.scalar_tensor_tensor(vv[:], x3d[:, :, 0], -8192.0,
                                           qc[:], op0=Alu.mult, op1=Alu.add)
            vbs = wrkb.tile([128, seg], dt.int32, tag="vbs")
            nc.vector.tensor_scalar(vbs[:], vv[:], 262144.0, None,
                                    op0=Alu.mult)
            # overflow (q>=4320): cast gives s|0x78 (fp8 Inf); +6 -> s|0x7E
            # (computed on the otherwise-idle Pool engine)
            t6 = wrkb.tile([128, seg], dt.int8, tag="t6")
            nc.vector.tensor_scalar(t6[:], q[:], 4320.0, 6.0,
                                    op0=Alu.is_ge, op1=Alu.mult)
            state[ci] = (vbs, t6)

        def stage3(ci):
            f32 = sizes[ci % n_chunks]
            seg = f32 // 32
            _, _, wrkb, out_pool = pools[f32]
            y_ap = y_flat[offs[ci % n_chunks] // 32:
                          offs[ci % n_chunks + 1] // 32]\
                .rearrange("(p f) -> p f", p=128)
            vbs, t6 = state.pop(ci)
            # fp8e4 RNE cast emits the byte s<<7|e<<3|m directly.
            # On DVE (not ACT) so the ACT queue holds only early-stage
            # copies and no engine queue mixes pipeline phases.
            f8 = wrkb.tile([128, seg], dt.float8e4, tag="f8")
            nc.vector.tensor_scalar(f8[:], vbs[:].bitcast(dt.float32),
                                    1.0, None, op0=Alu.mult)
            o_t = out_pool.tile([128, seg], dt.int8, tag="o")
            nc.vector.tensor_tensor(o_t[:], f8[:].bitcast(dt.int8), t6[:],
                                    op=Alu.add)
            if not compute_only or ci == repeat * n_chunks - 1:
                # Pool SWDGE keeps outs off the busy queues mid-stream, but
                # its ~1us serial descriptor gen would dominate the drain —
                # the last few outs go via the sync HWDGE queue, idle by then
                if ci >= repeat * n_chunks - 3:
                    nc.sync.dma_start(y_ap, o_t[:])
                else:
                    nc.gpsimd.dma_start(y_ap, o_t[:])

        total = repeat * n_chunks
        # software-pipelined emission: stage2 lags stage1 by D2 chunks and
        # stage3 by D3, so every cross-engine dependency is on data
        # computed >=2 chunk-periods earlier and no in-order queue blocks
        d2 = min(2, max(1, total - 1))
        d3 = min(4, max(d2 + 1, total - 1)) if total > 1 else 0
        for ci in range(total):
            stage1(ci)
            if ci >= d2:
                stage2(ci - d2)
            if ci >= d3:
                stage3(ci - d3)
        done3 = total - d3
        for ci in range(max(total - d2, 0), total):
            stage2(ci)
            if done3 < total:
                stage3(done3)
                done3 += 1
        while done3 < total:
            stage3(done3)
            done3 += 1

    nc.compile()
    return nc


_NC_CACHE = {}


def _get_nc(repeat: int = 1):
    if repeat not in _NC_CACHE:
        _NC_CACHE[repeat] = _build_program(repeat)
    return _NC_CACHE[repeat]


def run(fp32_pulse: np.ndarray, trace: bool = False):
    fp32_pulse = np.ascontiguousarray(np.asarray(fp32_pulse, dtype=np.float32))
    assert fp32_pulse.shape == (N_ROWS, 32), fp32_pulse.shape
    nc = _get_nc()
    shards = np.split(fp32_pulse, N_CORES, axis=0)
    in_maps = [{"x": s} for s in shards]
    res = run_bass_kernel_spmd(nc, in_maps, list(range(N_CORES)), trace=trace)
    packed = np.concatenate([r["y"].view(np.uint8) for r in res.results])
    out = np.unpackbits(packed).reshape(N_ROWS, 8).astype(np.float32)
    return out, res


def kernel(fp32_pulse: np.ndarray) -> np.ndarray:
    out, _ = run(fp32_pulse, trace=False)
    return out



# revision 4
# speedup vs baseline: 1.1474x; 1.1474x over previous
"""FP32 -> FP8 E4M3 bit-pulse converter on 8 Trainium2 NeuronCores.

Input : fp32_pulse [2097152, 32] float32 of 0/1 pulses, [S, E7..E0, M22..M0]
Output: [2097152, 8] float32 of 0/1 pulses, [S, E3..E0, M2..M0]

Strategy (per core, batch-sharded 8 ways):
  - q = 32*exp + 16*m22 + 8*m21 + 4*m20 + 2*m19 + sticky  (13-bit int) via a
    scalar_tensor_tensor MAC chain on DVE (fp32 in, bf16 intermediates).
  - sticky = OR of the 19 low mantissa bits: ACT copies them to a padded
    contiguous bf16 [s][20] tile, then DVE runs an all-tensor_tensor max
    tree (16->8, fold 4, 4->2->1) in the 2x-packed bf16 mode.
  - v = bitcast(int32((clamp(q,3712,8159) - 8192*s) * 2^18)): exactly the
    fp32 with sign = s, exponent field = exp, mantissa = m22..m19 |
    sticky<<18.  The underflow clamp maps exp<=116 to a value that rounds
    to zero; the 8159 cap keeps the exponent field <= 254 (no Inf/NaN).
  - A DVE fp32->fp8e4 cast performs the exact RNE (incl. subnormals) and
    emits the full byte s<<7|e<<3|m.  The converter is IEEE-style: finite
    overflow gives the Inf byte s|0x78, so (q>=4320)*6 is added to the
    byte to produce the reference's saturated s|0x7E.
  - The packed byte per row is DMA'd out (32x less write traffic than the
    8 fp32 pulses); the host unpacks bytes to the output pulses.
  - Work is spread so each in-order engine queue holds ops of a single
    pipeline phase (ACT: input copies; DVE: math + cast; Pool/SWDGE:
    mid-stream output DMAs; sync: input DMAs + the last few outputs) —
    otherwise cross-engine semaphore waits head-of-line-block the next
    chunk's early ops and the pipeline serializes at chain latency.
"""

from contextlib import ExitStack

import numpy as np

import concourse.bacc as bacc
import concourse.mybir as mybir
from concourse import tile
from concourse.bass_utils import run_bass_kernel_spmd

N_ROWS = 2097152
N_CORES = 8
ROWS_PER_CORE = N_ROWS // N_CORES          # 262144

dt = mybir.dt
Alu = mybir.AluOpType
Act = mybir.ActivationFunctionType

F32 = 4096                                  # fp32 per partition per chunk
SIZES = [4096] * 16
MAIN_BUFS = (6, 3, 4, 3)                    # xin, wrka, wrkb, out
TAIL_BUFS = (3, 2, 3, 3)
DMA_LEAD = 2                                # input-DMA prefetch distance


def _build_program(repeat: int = 1, rows_per_core: int = ROWS_PER_CORE,
                   f32: int = None, sizes=None, compute_only: bool = False,
                   main_bufs=MAIN_BUFS, tail_bufs=TAIL_BUFS,
                   dma_lead: int = DMA_LEAD):
    if sizes is None:
        if f32 is not None:
            sizes = [f32] * (rows_per_core * 32 // (128 * f32))
        else:
            sizes = list(SIZES)
    assert sum(sizes) * 128 == rows_per_core * 32
    n_chunks = len(sizes)
    main_f32 = max(sizes)

    nc = bacc.Bacc("TRN2", target_bir_lowering=False, debug=False,
                   num_devices=N_CORES)
    x_dram = nc.dram_tensor("x", [rows_per_core, 32], dt.float32,
                            kind="ExternalInput")
    y_dram = nc.dram_tensor("y", [rows_per_core], dt.int8,
                            kind="ExternalOutput")
    x_flat = x_dram.ap().rearrange("r w -> (r w)")
    y_flat = y_dram.ap()
    offs = [0]
    for sz in sizes:
        offs.append(offs[-1] + 128 * sz)

    with tile.TileContext(nc) as tc, ExitStack() as ps:
        pools = {}
        for sz in sorted(set(sizes), reverse=True):
            bs = main_bufs if sz == main_f32 else tail_bufs
            pools[sz] = tuple(
                ps.enter_context(tc.tile_pool(name=f"{nm}{sz}", bufs=b))
                for nm, b in zip(("xin", "wrka", "wrkb", "out"), bs))

        state = {}
        xf_state = {}
        xf_cache = {}

        def stage0(ci):
            # input DMA prefetch, alternating between the two HWDGE rings
            # (qSPDynamicHW / qActDynamicHW) so per-transfer completion
            # latency on one ring overlaps data flow on the other
            f32 = sizes[ci % n_chunks]
            xin_pool = pools[f32][0]
            x_ap = x_flat[offs[ci % n_chunks]:offs[ci % n_chunks + 1]]\
                .rearrange("(p f) -> p f", p=128)
            eng = nc.sync if ci % 2 == 0 else nc.scalar
            if compute_only:
                if f32 not in xf_cache:
                    xf = xin_pool.tile([128, f32], dt.float32, tag="xf")
                    eng.dma_start(xf[:], x_ap)
                    xf_cache[f32] = xf
                xf_state[ci] = xf_cache[f32]
            else:
                xf = xin_pool.tile([128, f32], dt.float32, tag="xf")
                eng.dma_start(xf[:], x_ap)
                xf_state[ci] = xf

        def stage1(ci):
            f32 = sizes[ci % n_chunks]
            seg = f32 // 32
            xin_pool, wrka, wrkb, out_pool = pools[f32]
            xf = xf_state.pop(ci)
            x3d = xf[:].rearrange("p (s c) -> p s c", c=32)

            # ACT: sticky cols 13..31 -> contiguous bf16 [s][20], col19 = 0
            stk = wrka.tile([128, seg * 20], dt.bfloat16, tag="stk")
            stkv = stk[:].rearrange("p (s c) -> p s c", c=20)
            nc.scalar.activation(stkv[:, :, 19], x3d[:, :, 0], Act.Copy,
                                 scale=0.0)
            nc.scalar.copy(stkv[:, :, 0:19], x3d[:, :, 13:32])

            # DVE: MAC over cols 1..12 (values stay exact in bf16 <= 255)
            y1 = wrka.tile([128, seg * 6], dt.bfloat16, tag="y1")
            y1_3d = y1[:].rearrange("p (s k) -> p s k", k=6)
            nc.vector.scalar_tensor_tensor(y1_3d[:], x3d[:, :, 1:12:2],
                                           2.0, x3d[:, :, 2:13:2],
                                           op0=Alu.mult, op1=Alu.add)
            y2 = wrka.tile([128, seg * 3], dt.bfloat16, tag="y2")
            y2_3d = y2[:].rearrange("p (s k) -> p s k", k=3)
            nc.vector.scalar_tensor_tensor(y2_3d[:], y1_3d[:, :, 0::2],
                                           4.0, y1_3d[:, :, 1::2],
                                           op0=Alu.mult, op1=Alu.add)
            r1 = wrka.tile([128, seg], dt.bfloat16, tag="r1")
            nc.vector.scalar_tensor_tensor(r1[:], y2_3d[:, :, 0], 16.0,
                                           y2_3d[:, :, 1],
                                           op0=Alu.mult, op1=Alu.add)
            t_t = wrka.tile([128, seg], dt.float32, tag="t")
            nc.vector.scalar_tensor_tensor(t_t[:], r1[:], 16.0,
                                           y2_3d[:, :, 2],
                                           op0=Alu.mult, op1=Alu.add)
            state[ci] = (x3d, stkv, t_t)

        def stage2(ci):
            f32 = sizes[ci % n_chunks]
            seg = f32 // 32
            _, wrka, wrkb, _ = pools[f32]
            x3d, stkv, t_t = state[ci]
            # DVE sticky tree, all stages 2x-packed bf16 and 4B-aligned:
            # 20 -> 8 (cols 0..15), fold cols 16..19, 4 -> 2 -> 1
            sa = wrka.tile([128, seg * 8], dt.bfloat16, tag="sa")
            sav = sa[:].rearrange("p (s c) -> p s c", c=8)
            nc.vector.tensor_tensor(sav[:], stkv[:, :, 0:8],
                                    stkv[:, :, 8:16], op=Alu.max)
            sb = wrkb.tile([128, seg * 4], dt.bfloat16, tag="sb")
            sbv = sb[:].rearrange("p (s c) -> p s c", c=4)
            nc.vector.tensor_tensor(sbv[:], sav[:, :, 0:4],
                                    sav[:, :, 4:8], op=Alu.max)
            sc = wrkb.tile([128, seg * 4], dt.bfloat16, tag="sc")
            scv = sc[:].rearrange("p (s c) -> p s c", c=4)
            nc.vector.tensor_tensor(scv[:], sbv[:], stkv[:, :, 16:20],
                                    op=Alu.max)
            sd = wrkb.tile([128, seg * 2], dt.bfloat16, tag="sd")
            sdv = sd[:].rearrange("p (s c) -> p s c", c=2)
            nc.vector.tensor_tensor(sdv[:], scv[:, :, 0:2], scv[:, :, 2:4],
                                    op=Alu.max)
            red = wrkb.tile([128, seg], dt.bfloat16, tag="red")
            nc.vector.tensor_tensor(red[:], sdv[:, :, 0], sdv[:, :, 1],
                                    op=Alu.max)

            q = wrkb.tile([128, seg], dt.float32, tag="q")
            nc.vector.scalar_tensor_tensor(q[:], t_t[:], 2.0, red[:],
                                           op0=Alu.mult, op1=Alu.add)
            # clamp: underflow to 3712 (rounds to 0); cap at 8159 so the
            # fp32 exponent field stays <= 254 (no Inf/NaN inputs — large
            # finite values cast to the fp8 Inf byte 0x78, fixed up below)
            qc = wrkb.tile([128, seg], dt.float32, tag="qc")
            nc.vector.tensor_scalar(qc[:], q[:], 8159.0, 3712.0,
                                    op0=Alu.min, op1=Alu.max)
            # fold the sign pulse: vv = q' - 8192*s, so vv<<18 has bit31 = s
            vv = wrkb.tile([128, seg], dt.float32, tag="vv")
            nc.vector.scalar_tensor_tensor(vv[:], x3d[:, :, 0], -8192.0,
                                           qc[:], op0=Alu.mult, op1=Alu.add)
            vbs = wrkb.tile([128, seg], dt.int32, tag="vbs")
            nc.vector.tensor_scalar(vbs[:], vv[:], 262144.0, None,
                                    op0=Alu.mult)
            # overflow (q>=4320): cast gives s|0x78 (fp8 Inf); +6 -> s|0x7E
            # (computed on the otherwise-idle Pool engine)
            t6 = wrkb.tile([128, seg], dt.int8, tag="t6")
            nc.vector.tensor_scalar(t6[:], q[:], 4320.0, 6.0,
                                    op0=Alu.is_ge, op1=Alu.mult)
            state[ci] = (vbs, t6)

        def stage3(ci):
            f32 = sizes[ci % n_chunks]
            seg = f32 // 32
            _, _, wrkb, out_pool = pools[f32]
            y_ap = y_flat[offs[ci % n_chunks] // 32:
                          offs[ci % n_chunks + 1] // 32]\
                .rearrange("(p f) -> p f", p=128)
            vbs, t6 = state.pop(ci)
            # fp8e4 RNE cast emits the byte s<<7|e<<3|m directly.
            # On DVE (not ACT) so the ACT queue holds only early-stage
            # copies and no engine queue mixes pipeline phases.
            f8 = wrkb.tile([128, seg], dt.float8e4, tag="f8")
            nc.vector.tensor_scalar(f8[:], vbs[:].bitcast(dt.float32),
                                    1.0, None, op0=Alu.mult)
            o_t = out_pool.tile([128, seg], dt.int8, tag="o")
            nc.vector.tensor_tensor(o_t[:], f8[:].bitcast(dt.int8), t6[:],
                                    op=Alu.add)
            if not compute_only or ci == repeat * n_chunks - 1:
                # Pool SWDGE keeps outs off the busy queues mid-stream, but
                # its ~1us serial descriptor gen would dominate the drain —
                # the last few outs go via the sync HWDGE queue, idle by then
                if ci >= repeat * n_chunks - 3:
                    nc.sync.dma_start(y_ap, o_t[:])
                else:
                    nc.gpsimd.dma_start(y_ap, o_t[:])

        total = repeat * n_chunks
        # software-pipelined emission: input DMA leads stage1 by dma_lead
        # chunks, stage2 lags stage1 by D2 and stage3 by D3, so every
        # cross-engine dependency is on data computed >=2 chunk-periods
        # earlier and no in-order queue blocks
        d0 = min(dma_lead, max(total - 1, 0))
        d2 = min(2, max(1, total - 1))
        d3 = min(4, max(d2 + 1, total - 1)) if total > 1 else 0
        for ci in range(d0):
            stage0(ci)
        for ci in range(total):
            if ci + d0 < total:
                stage0(ci + d0)
            stage1(ci)
            if ci >= d2:
                stage2(ci - d2)
            if ci >= d3:
                stage3(ci - d3)
        done3 = total - d3
        for ci in range(max(total - d2, 0), total):
            stage2(ci)
            if done3 < total:
                stage3(done3)
                done3 += 1
        while done3 < total:
            stage3(done3)
            done3 += 1

    nc.compile()
    return nc


_NC_CACHE = {}


def _get_nc(repeat: int = 1):
    if repeat not in _NC_CACHE:
        _NC_CACHE[repeat] = _build_program(repeat)
    return _NC_CACHE[repeat]


def run(fp32_pulse: np.ndarray, trace: bool = False):
    fp32_pulse = np.ascontiguousarray(np.asarray(fp32_pulse, dtype=np.float32))
    assert fp32_pulse.shape == (N_ROWS, 32), fp32_pulse.shape
    nc = _get_nc()
    shards = np.split(fp32_pulse, N_CORES, axis=0)
    in_maps = [{"x": s} for s in shards]
    res = run_bass_kernel_spmd(nc, in_maps, list(range(N_CORES)), trace=trace)
    packed = np.concatenate([r["y"].view(np.uint8) for r in res.results])
    out = np.unpackbits(packed).reshape(N_ROWS, 8).astype(np.float32)
    return out, res


def kernel(fp32_pulse: np.ndarray) -> np.ndarray:
    out, _ = run(fp32_pulse, trace=False)
    return out

